# revision 22
# baseline (speedup 1.0000x reference)
"""Trainium2 Bass kernel for nn_CustomTransformer2D (gnn message passing).

Math (validated against the reference in fp64):
  - The q/LN1/Wq branch only shifts attention logits by a constant along the
    softmax axis M, so it cancels in softmax; q enters only via the residual.
  - bk and bp are constant along M too -> dropped from the logits.
  - v = Wv@(Wk@k+bk)+bv = k@(Wv@Wk)^T + bkv;  since sum_m a = 1, the bkv
    offset commutes through the attention sum and is folded into Wo's bias.
  - ln2 gain/bias are folded into W1/b1.

Per-core layout (batch b on core b): tokens on SBUF partitions; the matmul
path runs in bf16 (inputs cast during the SWDGE DMA), accumulation f32.

Per chunk of 128 tokens: k tile (2-chunk batched loads) --PE transpose per
m-pair--> TS; S/v matmuls stream WSV per transposed tile into a
(bank, S|v, pair, parity, d) PSUM layout so the pos matmuls land as 4
contiguous 256-col accumulations.  Softmax: exp on ACT (bf16 out), e*v on
DVE; all small downstream ops run at 2-chunk granularity to amortize
per-instruction overheads: single strided reduces for the m-sums, softmax
normalize via one GpSimd divide, LN2 rstd via exp(-0.5*ln(var+eps)) on ACT
(exp/ln/copy share one table set), MLP first layer computed directly in
transposed form (weights stationary, bias via the ones-row trick), relu on
DVE, second layer straight from the transposed hidden tile.
"""

import numpy as np

B, N, M, D, F = 8, 16384, 16, 64, 256
EPS = 1e-5
CHUNK = 128  # tokens per chunk

_prog_cache = {}


def _bf16(x):
    import ml_dtypes

    return np.asarray(x).astype(ml_dtypes.bfloat16)


def _prep_weights(inp):
    f = np.float32
    Wk, bk = inp["Wk"], inp["bk"]
    Wv, bv = inp["Wv"], inp["bv"]
    Wp = inp["Wp"]
    Wo, bo = inp["Wo"], inp["bo"]
    W1, b1 = inp["W1"], inp["b1"]
    W2, b2 = inp["W2"], inp["b2"]
    g2, bl2 = inp["ln2_g"], inp["ln2_b"]

    Wkv = (Wv @ Wk).astype(f)
    bkv = (Wv @ bk + bv).astype(f)
    bo_p = (bo + Wo @ bkv).astype(f)
    W1p = (W1 * g2[None, :]).astype(f)
    b1p = (b1 + W1 @ bl2).astype(f)

    wsv = np.zeros((128, 256), f)
    wsv[0:64, 0:64] = Wk.T
    wsv[64:128, 64:128] = Wk.T
    wsv[0:64, 128:192] = Wkv.T
    wsv[64:128, 192:256] = Wkv.T

    # pos weights, S columns only.  Per half h and bank bk the pos matmul
    # accumulates a contiguous 256-col block (jm, r, d) onto the S region;
    # block index = 2*h + bk, pair j = 4*h + 2*bk + jm, m = 2*j + r.
    # Duplicated on partition halves so the stationary PTS2 half (base
    # partition 0 or 64) matches the rhs base partition.
    wps = np.zeros((128, 1024), f)
    for blk in range(4):
        for jm in range(2):
            for r in range(2):
                m = 2 * (2 * blk + jm) + r
                for par in range(2):
                    rows = slice(64 * par + 4 * m, 64 * par + 4 * m + 4)
                    cols = slice(
                        256 * blk + 128 * jm + 64 * r,
                        256 * blk + 128 * jm + 64 * r + 64,
                    )
                    wps[rows, cols] = Wp.T

    woe = np.zeros((65, 64), f)
    woe[0:64] = Wo.T
    woe[64] = bo_p

    w1e = np.zeros((65, 256), f)
    w1e[0:64] = W1p.T
    w1e[64] = b1p

    w2s = np.concatenate([W2.T[0:128], W2.T[128:256]], axis=1).astype(f)  # [128,128]

    return {
        "wsv": _bf16(wsv),
        "wps": _bf16(wps),
        "woe": _bf16(woe),
        "w1e": _bf16(w1e),
        "w2s": _bf16(w2s),
        "b2c": b2.astype(f),
        "ident": _bf16(np.eye(128, dtype=f)),
        "ones": _bf16(np.ones(128, f)),
    }


def _patch_tile_drain():
    """This container's walrus build rejects instructions with more than one
    sync-wait command. Tile's kernel-tail drain carries one wait per logical
    processor; split them across sync-engine nops."""
    import concourse.tile as tile
    from concourse.vector_clock import ScopedClock, VectorClock

    if getattr(tile.TileContext, "_ant_drain_patched", False):
        return

    def _drain_and_barrier(self, tick_clock, wait_clock):
        nc = self.nc
        gc = tick_clock.global_clock
        n = len(gc)
        procs = [i for i in range(n) if gc[i] > 0]
        for p in procs:
            sub = VectorClock([gc[j] if j == p else 0 for j in range(n)])
            nop = nc.sync.nop(nofuse=True, hint="drain_split")
            wait_clock.add_sem_waits(nop.ins, ScopedClock({None: sub}))
        nc.sync.drain()
        nc.all_engine_barrier()
        popped = nc._tile_sem_poison_stack.pop()
        assert popped is self._sem_poison
        nc.clear_and_free_semaphores(list(self.sems.allocated().values()))
        nc.all_engine_barrier()

    tile.TileContext._drain_and_barrier = _drain_and_barrier
    tile.TileContext._ant_drain_patched = True


def _split_multi_waits(nc):
    """Hoist extra sync waits onto same-engine NoOps inserted right before
    the instruction (the engine stalls at the nop, semantics unchanged)."""
    import bass_rust
    import concourse.mybir as mybir

    k = 0
    for blk in nc.main_func.blocks:
        insts = blk.instructions
        need = False
        for ins in insts:
            si = ins.sync_info
            if si is not None and len(si.on_wait) > 1:
                need = True
                break
        if not need:
            continue
        out = []
        for ins in insts:
            si = ins.sync_info
            if (
                si is not None
                and len(si.on_wait) > 1
                and ins.engine != mybir.EngineType.Unassigned
            ):
                waits = list(si.on_wait)
                for w in waits[:-1]:
                    k += 1
                    nop = mybir.InstNoOp(
                        name=f"wsplit-{k}", ins=[], outs=[], engine=ins.engine
                    )
                    nop.sync_info = bass_rust.SyncInfo(on_wait=[w], on_update=[])
                    nc.register_instruction(nop, overwrite=True)
                    out.append(nop)
                ins.sync_info = bass_rust.SyncInfo(
                    on_wait=[waits[-1]], on_update=list(si.on_update)
                )
            out.append(ins)
        blk.instructions = out


def build_program(n_tokens):
    """Build the per-core Bass program (same for every core)."""
    import concourse.bass as bass
    import concourse.tile as tile
    import concourse.mybir as mybir

    _patch_tile_drain()

    dt = mybir.dt
    f32 = dt.float32
    bf16 = dt.bfloat16
    Alu = mybir.AluOpType
    Act = mybir.ActivationFunctionType

    nc = bass.Bass(trn_type="TRN2")

    k_d = nc.dram_tensor("k", [n_tokens, M * D], f32, kind="ExternalInput")
    pos_d = nc.dram_tensor("pos", [n_tokens, M * 4], f32, kind="ExternalInput")
    q_d = nc.dram_tensor("q", [n_tokens, D], f32, kind="ExternalInput")
    wsv_d = nc.dram_tensor("wsv", [128, 256], bf16, kind="ExternalInput")
    wps_d = nc.dram_tensor("wps", [128, 1024], bf16, kind="ExternalInput")
    woe_d = nc.dram_tensor("woe", [65, 64], bf16, kind="ExternalInput")
    w1e_d = nc.dram_tensor("w1e", [65, 256], bf16, kind="ExternalInput")
    w2s_d = nc.dram_tensor("w2s", [128, 128], bf16, kind="ExternalInput")
    b2c_d = nc.dram_tensor("b2c", [D], f32, kind="ExternalInput")
    ident_d = nc.dram_tensor("ident", [128, 128], bf16, kind="ExternalInput")
    ones_d = nc.dram_tensor("ones", [128], bf16, kind="ExternalInput")
    out_d = nc.dram_tensor("out", [n_tokens, D], f32, kind="ExternalOutput")

    nchunks = n_tokens // CHUNK
    npairs = nchunks // 2
    assert n_tokens % (2 * CHUNK) == 0

    with tile.TileContext(nc) as tc:
        with (
            tc.tile_pool(name="singles", bufs=1) as singles,
            tc.tile_pool(name="kin", bufs=3) as kin,
            tc.tile_pool(name="small_in", bufs=6) as small_in,
            tc.tile_pool(name="ts", bufs=4) as tsp,
            tc.tile_pool(name="ev", bufs=4) as evp_pool,
            tc.tile_pool(name="lnw", bufs=8) as lnw,
            tc.tile_pool(name="outp", bufs=6) as outp,
            tc.tile_pool(name="sp", bufs=2, space="PSUM") as sp_pool,
            tc.tile_pool(name="tp_ps", bufs=2, space="PSUM") as tp_ps,
            tc.tile_pool(name="tail_ps", bufs=2, space="PSUM") as tail_ps,
        ):
            # constants
            WSV = singles.tile([128, 256], bf16)
            WPS = singles.tile([128, 1024], bf16)
            WOE = singles.tile([65, 64], bf16)
            W1E = singles.tile([65, 256], bf16)
            W2S = singles.tile([128, 128], bf16)
            B2B = singles.tile([128, 2, D], f32)
            IDENT = singles.tile([128, 128], bf16)
            EPST = singles.tile([128, 1], f32)
            nc.vector.memset(EPST[:], EPS)
            OTSX = [
                singles.tile([65, 128], bf16, tag=f"otsx{i}", name=f"OTSX{i}")
                for i in range(4)
            ]
            HTSX = [
                singles.tile([65, 128], bf16, tag=f"htsx{i}", name=f"HTSX{i}")
                for i in range(4)
            ]
            for t in OTSX:
                nc.sync.dma_start(out=t[64:65, :], in_=ones_d[:])
            for t in HTSX:
                nc.sync.dma_start(out=t[64:65, :], in_=ones_d[:])
            nc.sync.dma_start(out=WSV[:], in_=wsv_d[:])
            nc.sync.dma_start(out=WPS[:], in_=wps_d[:])
            nc.sync.dma_start(out=WOE[:], in_=woe_d[:])
            nc.sync.dma_start(out=W1E[:], in_=w1e_d[:])
            nc.sync.dma_start(out=W2S[:], in_=w2s_d[:])
            nc.sync.dma_start(out=IDENT[:], in_=ident_d[:])
            b2_bcast = bass.AP(
                tensor=b2c_d[:].tensor, offset=0, ap=[[0, 128], [0, 2], [1, D]]
            )
            nc.gpsimd.dma_start(out=B2B[:], in_=b2_bcast)

            fstate = {}

            def front(ci, E2, EV2):
                """Front stage for one 128-token chunk ci; writes its share
                of the pair tiles E2/EV2 (cc = ci%2)."""
                n0 = ci * CHUNK
                cc = ci % 2
                if cc == 0:
                    # 2-chunk batched loads (cast f32->bf16 during DMA)
                    KD2 = kin.tile([128, 2, M * D], bf16, tag="kd")
                    nc.gpsimd.dma_start(
                        out=KD2[:],
                        in_=k_d[n0 : n0 + 256, :].rearrange(
                            "(c p) e -> p c e", c=2, p=128
                        ),
                    )
                    PD2 = small_in.tile([128, 2, M * 4], bf16, tag="pd")
                    nc.gpsimd.dma_start(
                        out=PD2[:],
                        in_=pos_d[n0 : n0 + 256, :].rearrange(
                            "(c p) e -> p c e", c=2, p=128
                        ),
                    )
                    PT2 = tp_ps.tile([128, 128], bf16, tag="tp")
                    nc.tensor.transpose(
                        PT2[:], PD2[:].rearrange("p c e -> p (c e)"), IDENT[:]
                    )
                    PTS2 = tsp.tile([128, 128], bf16, tag="pts")
                    nc.scalar.copy(out=PTS2[:], in_=PT2[:])
                    fstate["KD2"] = KD2
                    fstate["PTS2"] = PTS2
                KD2 = fstate["KD2"]
                PTS2 = fstate["PTS2"]

                # transposed k tiles via PE into PSUM; evacuate per half so
                # the S/v matmuls of half 0 start while half 1 transposes
                TS = tsp.tile([128, M * D], bf16, tag="ts")
                # half 0: tiles j0-j2 transposed by the idle sync DMA ring
                # (xbar), j3 by PE; half 1: all PE.  Spreads the transpose
                # work across PE, ACT (evacuation copies) and the DMA ring.
                for j in range(3):
                    nc.sync.dma_start(
                        out=TS[:, 128 * j : 128 * (j + 1)],
                        in_=KD2[:, cc, 128 * j : 128 * (j + 1)],
                        transpose=True,
                    )
                TP0 = tp_ps.tile([128, 128], bf16, tag="tp")
                nc.tensor.transpose(TP0[:], KD2[:, cc, 384:512], IDENT[:])
                nc.scalar.copy(out=TS[:, 384:512], in_=TP0[:])
                TPh = tp_ps.tile([128, 512], bf16, tag="tp")
                for jj in range(4):
                    nc.tensor.transpose(
                        TPh[:, 128 * jj : 128 * (jj + 1)],
                        KD2[:, cc, 512 + 128 * jj : 512 + 128 * (jj + 1)],
                        IDENT[:],
                    )
                nc.scalar.copy(out=TS[:, 512:1024], in_=TPh[:])

                for h in range(2):
                    # [t, bank, S|v, pair, parity, d]
                    SPh = sp_pool.tile([128, 2, 2, 2, 2, D], f32, tag="sp")
                    for j in range(4):
                        bk, jm = j // 2, j % 2
                        nc.tensor.matmul(
                            SPh[:, bk, :, jm, :, :],
                            TS[:, 128 * (4 * h + j) : 128 * (4 * h + j + 1)],
                            WSV[:],
                            start=(jm == 0),
                            stop=False,
                        )
                    for bk in range(2):
                        blk = 2 * h + bk
                        nc.tensor.matmul(
                            SPh[:, bk, 0, :, :, :],
                            PTS2[64 * cc : 64 * cc + 64, :],
                            WPS[64 * cc : 64 * cc + 64, 256 * blk : 256 * (blk + 1)],
                            start=False,
                            stop=True,
                        )
                    nc.scalar.activation(
                        out=E2[:, cc, h, :],
                        in_=SPh[:, :, 0, :, :, :].rearrange("p b j r d -> p d b j r"),
                        func=Act.Exp,
                    )
                    nc.vector.tensor_mul(
                        EV2[:, cc, h, :],
                        E2[:, cc, h, :],
                        SPh[:, :, 1, :, :, :].rearrange("p b j r d -> p d b j r"),
                    )

            def back1(pi, E2, EV2, QD2):
                # m-sums for both chunks of the pair; free layout (c, d)
                SM2 = lnw.tile([128, 2, D], f32, tag="sm")
                nc.vector.tensor_reduce(
                    out=SM2[:],
                    in_=E2[:].rearrange("p c h (d m) -> p c d h m", d=D, m=8),
                    axis=mybir.AxisListType.XY,
                    op=Alu.add,
                )
                OV1 = lnw.tile([128, 2, 512], bf16, tag="ov1")
                nc.gpsimd.tensor_add(OV1[:], EV2[:, :, 0, :], EV2[:, :, 1, :])
                OV1v = OV1[:].rearrange("p c (d m) -> p c d m", d=D, m=8)
                OV2a = lnw.tile([128, 2, D, 4], bf16, tag="ov2a")
                nc.gpsimd.tensor_add(OV2a[:], OV1v[:, :, :, 0:4], OV1v[:, :, :, 4:8])
                OV2 = lnw.tile([128, 2, D], f32, tag="ov")
                nc.vector.tensor_reduce(
                    out=OV2[:],
                    in_=OV2a[:],
                    axis=mybir.AxisListType.X,
                    op=Alu.add,
                )
                LS2 = lnw.tile([128, 2, D], f32, tag="ls")
                nc.scalar.activation(out=LS2[:], in_=SM2[:], func=Act.Ln)
                RC2 = lnw.tile([128, 2, D], f32, tag="rc")
                nc.scalar.activation(out=RC2[:], in_=LS2[:], func=Act.Exp, scale=-1.0)
                OA2 = lnw.tile([128, 2, D], bf16, tag="oa")
                nc.gpsimd.tensor_mul(OA2[:], OV2[:], RC2[:])
                return OA2, QD2

            def back2(pi, OA2, QD2):
                # Wo + residual for both chunks
                OT2 = tail_ps.tile([128, 128], bf16, tag="tlps")
                nc.tensor.transpose(
                    OT2[:], OA2[:].rearrange("p c d -> p (c d)"), IDENT[:]
                )
                OO2 = tail_ps.tile([128, 2, D], f32, tag="tlps")
                for cc in range(2):
                    OTS = OTSX[(2 * pi + cc) % 4]
                    nc.scalar.copy(out=OTS[0:64, :], in_=OT2[64 * cc : 64 * cc + 64, :])
                    nc.tensor.matmul(
                        OO2[:, cc, :], OTS[:], WOE[:], start=True, stop=True
                    )
                R12 = lnw.tile([128, 2, D], f32, tag="r1")
                nc.vector.tensor_add(R12[:], QD2[:], OO2[:])
                RB2 = lnw.tile([128, 2, D], f32, tag="rb")
                nc.gpsimd.tensor_add(RB2[:], R12[:], B2B[:])

                # LN2 stats per chunk; rstd = exp(-0.5*ln(var+eps)), batched
                MVA = lnw.tile([128, 2, 2], f32, tag="mva")
                for cc in range(2):
                    ST6 = lnw.tile([128, 6], f32, tag="st6")
                    nc.vector.bn_stats(out=ST6[:], in_=R12[:, cc, :])
                    nc.vector.bn_aggr(out=MVA[:, cc, :], in_=ST6[:])
                LV2 = lnw.tile([128, 2], f32, tag="lv")
                nc.scalar.activation(
                    out=LV2[:], in_=MVA[:, :, 1], func=Act.Ln, bias=EPST[:]
                )
                YA2 = lnw.tile([128, 2], f32, tag="ya")
                nc.scalar.activation(out=YA2[:], in_=LV2[:], func=Act.Exp, scale=-0.5)
                CT2 = lnw.tile([128, 2, D], bf16, tag="ct")
                for cc in range(2):
                    nc.vector.tensor_scalar(
                        out=CT2[:, cc, :],
                        in0=R12[:, cc, :],
                        scalar1=MVA[:, cc, 0:1],
                        scalar2=YA2[:, cc : cc + 1],
                        op0=Alu.subtract,
                        op1=Alu.mult,
                    )
                return CT2, RB2

            def back3(pi, CT2, RB2):
                n0 = 2 * pi * CHUNK
                # MLP (bf16); first layer computed directly transposed:
                # H1T[f,t] with W1 halves stationary, bias via ones row.
                HT2 = tail_ps.tile([128, 128], bf16, tag="tlps")
                nc.tensor.transpose(
                    HT2[:], CT2[:].rearrange("p c d -> p (c d)"), IDENT[:]
                )
                H1S = lnw.tile([128, 2, 2, 128], bf16, tag="h1s")
                H1T = [
                    tail_ps.tile([128, 2, 128], f32, tag="tlps", name=f"H1T{fh}")
                    for fh in range(2)
                ]
                for cc in range(2):
                    HTS = HTSX[(2 * pi + cc) % 4]
                    nc.scalar.copy(
                        out=HTS[0:64, :], in_=HT2[64 * cc : 64 * cc + 64, :]
                    )
                    for fh in range(2):
                        nc.tensor.matmul(
                            H1T[fh][:, cc, :],
                            W1E[:, 128 * fh : 128 * (fh + 1)],
                            HTS[:],
                            start=True,
                            stop=True,
                        )
                for fh in range(2):
                    nc.vector.tensor_scalar_max(
                        H1S[:, :, fh, :], H1T[fh][:], 0.0
                    )
                H22 = tail_ps.tile([128, 2, D], f32, tag="tlps")
                for cc in range(2):
                    for fh in range(2):
                        nc.tensor.matmul(
                            H22[:, cc, :],
                            H1S[:, cc, fh, :],
                            W2S[:, 64 * fh : 64 * fh + 64],
                            start=(fh == 0),
                            stop=(fh == 1),
                        )
                OUTT = outp.tile([128, 2, D], f32, tag="outt")
                nc.vector.tensor_add(OUTT[:], H22[:], RB2[:])
                nc.sync.dma_start(
                    out=out_d[n0 : n0 + 256, :].rearrange(
                        "(c p) d -> p c d", c=2, p=128
                    ),
                    in_=OUTT[:],
                )

            def load_front(pi):
                E2 = evp_pool.tile([128, 2, 2, 512], bf16, tag="e")
                EV2 = evp_pool.tile([128, 2, 2, 512], bf16, tag="evp")
                front(2 * pi, E2, EV2)
                front(2 * pi + 1, E2, EV2)
                QD2 = small_in.tile([128, 2, D], f32, tag="qd")
                nc.sync.dma_start(
                    out=QD2[:],
                    in_=q_d[2 * pi * CHUNK : 2 * pi * CHUNK + 256, :].rearrange(
                        "(c p) d -> p c d", c=2, p=128
                    ),
                )
                return E2, EV2, QD2

            L1, L2, L3 = 2, 1, 1
            pend1 = {}
            pend2 = {}
            pend3 = {}
            for pi in range(npairs + L1 + L2 + L3):
                if pi < npairs:
                    pend1[pi] = load_front(pi)
                c = pi - L1
                if 0 <= c < npairs:
                    pend2[c] = back1(c, *pend1.pop(c))
                c = pi - L1 - L2
                if 0 <= c < npairs:
                    pend3[c] = back2(c, *pend2.pop(c))
                c = pi - L1 - L2 - L3
                if 0 <= c < npairs:
                    back3(c, *pend3.pop(c))

    _split_multi_waits(nc)
    return nc


def _get_program(n_tokens):
    if n_tokens not in _prog_cache:
        _prog_cache[n_tokens] = build_program(n_tokens)
    return _prog_cache[n_tokens]


def make_in_maps(inputs):
    """Shard full inputs into 8 per-core input maps."""
    w = _prep_weights(inputs)
    k = np.ascontiguousarray(inputs["k"])
    pos = np.ascontiguousarray(inputs["pos"])
    q = np.ascontiguousarray(inputs["q"])
    nt = k.shape[1]
    in_maps = []
    for b in range(B):
        in_maps.append(
            {
                "k": k[b].reshape(nt, M * D),
                "pos": pos[b].reshape(nt, M * 4),
                "q": q[b].reshape(nt, D),
                **w,
            }
        )
    return in_maps


LAST_EXEC_NS = None
LAST_RESULT = None


def _install_cc_probe():
    import subprocess
    import traceback

    import libneuronxla

    if getattr(libneuronxla, "_ant_probe", False):
        return
    shim = libneuronxla.neuronx_cc

    def loud(code, *a, **k):
        try:
            return shim(code, *a, **k)
        except subprocess.CalledProcessError as e:
            with open("/tmp/walrus_err.log", "w") as fh:
                fh.write(str(e.output))
            raise
        except BaseException:
            with open("/tmp/walrus_err.log", "w") as fh:
                fh.write(traceback.format_exc())
            raise

    libneuronxla.neuronx_cc = loud
    libneuronxla._ant_probe = True
    import concourse.bass2jax as b2j

    b2j.install_neuronx_cc_hook = lambda: None


def _ensure_ntff_hook():
    """Register the NTFF profiling hook if the image's antenv lacks it."""
    import sys
    import types

    try:
        from antenv.axon_hooks import get_axon_ntff_profile_hook  # noqa: F401

        return
    except ImportError:
        pass
    try:
        from trn_agent_boot.trn_boot import _ntff_profile_via_ctypes

        hook = _ntff_profile_via_ctypes("/opt/axon/libaxon_pjrt.so")
    except Exception:
        hook = None
    mod = types.ModuleType("antenv.axon_hooks")
    mod.get_axon_ntff_profile_hook = lambda: hook
    mod.set_axon_ntff_profile_hook = lambda h: None
    import antenv

    sys.modules["antenv.axon_hooks"] = mod
    antenv.axon_hooks = mod


def kernel(**inputs):
    global LAST_EXEC_NS, LAST_RESULT
    import os

    from concourse import bass_utils

    _install_cc_probe()
    trace = bool(int(os.environ.get("KERNEL_TRACE", "0")))
    if trace:
        _ensure_ntff_hook()
    nt = np.ascontiguousarray(inputs["k"]).shape[1]
    nc = _get_program(nt)
    in_maps = make_in_maps(inputs)
    res = bass_utils.run_bass_kernel_spmd(
        nc, in_maps, core_ids=list(range(B)), trace=trace
    )
    LAST_EXEC_NS = res.exec_time_ns
    LAST_RESULT = res
    out = np.stack([res.results[b]["out"].reshape(nt, D) for b in range(B)])
    return out.astype(np.float32)


# revision 24
# speedup vs baseline: 1.9498x; 1.9498x over previous
"""Trainium2 Bass kernel for nn_CustomTransformer2D (gnn message passing).

Math (validated against the reference in fp64):
  - The q/LN1/Wq branch only shifts attention logits by a constant along the
    softmax axis M, so it cancels in softmax; q enters only via the residual.
  - bk and bp are constant along M too -> dropped from the logits.
  - v = Wv@(Wk@k+bk)+bv = k@(Wv@Wk)^T + bkv;  since sum_m a = 1, the bkv
    offset commutes through the attention sum and is folded into Wo's bias.
  - ln2 gain/bias are folded into W1/b1.

Per-core layout (batch b on core b): tokens on SBUF partitions; the matmul
path runs in bf16 (inputs cast during the SWDGE DMA), accumulation f32.

Per chunk of 128 tokens: k tile (2-chunk batched loads) --PE transpose per
m-pair--> TS; S/v matmuls stream WSV per transposed tile into a
(bank, S|v, pair, parity, d) PSUM layout so the pos matmuls land as 4
contiguous 256-col accumulations.  Softmax: exp on ACT (bf16 out), e*v on
DVE; all small downstream ops run at 2-chunk granularity to amortize
per-instruction overheads: single strided reduces for the m-sums, softmax
normalize via one GpSimd divide, LN2 rstd via exp(-0.5*ln(var+eps)) on ACT
(exp/ln/copy share one table set), MLP first layer computed directly in
transposed form (weights stationary, bias via the ones-row trick), relu on
DVE, second layer straight from the transposed hidden tile.
"""

import numpy as np

B, N, M, D, F = 8, 16384, 16, 64, 256
EPS = 1e-5
CHUNK = 128  # tokens per chunk

_prog_cache = {}


def _bf16(x):
    import ml_dtypes

    return np.asarray(x).astype(ml_dtypes.bfloat16)


def _prep_weights(inp):
    f = np.float32
    Wk, bk = inp["Wk"], inp["bk"]
    Wv, bv = inp["Wv"], inp["bv"]
    Wp = inp["Wp"]
    Wo, bo = inp["Wo"], inp["bo"]
    W1, b1 = inp["W1"], inp["b1"]
    W2, b2 = inp["W2"], inp["b2"]
    g2, bl2 = inp["ln2_g"], inp["ln2_b"]

    Wkv = (Wv @ Wk).astype(f)
    bkv = (Wv @ bk + bv).astype(f)
    bo_p = (bo + Wo @ bkv).astype(f)
    W1p = (W1 * g2[None, :]).astype(f)
    b1p = (b1 + W1 @ bl2).astype(f)

    wsv = np.zeros((128, 256), f)
    wsv[0:64, 0:64] = Wk.T
    wsv[64:128, 64:128] = Wk.T
    wsv[0:64, 128:192] = Wkv.T
    wsv[64:128, 192:256] = Wkv.T

    # pos weights, S columns only.  Per half h and bank bk the pos matmul
    # accumulates a contiguous 256-col block (jm, r, d) onto the S region;
    # block index = 2*h + bk, pair j = 4*h + 2*bk + jm, m = 2*j + r.
    # Duplicated on partition halves so the stationary PTS2 half (base
    # partition 0 or 64) matches the rhs base partition.
    wps = np.zeros((128, 1024), f)
    for blk in range(4):
        for jm in range(2):
            for r in range(2):
                m = 2 * (2 * blk + jm) + r
                for par in range(2):
                    rows = slice(64 * par + 4 * m, 64 * par + 4 * m + 4)
                    cols = slice(
                        256 * blk + 128 * jm + 64 * r,
                        256 * blk + 128 * jm + 64 * r + 64,
                    )
                    wps[rows, cols] = Wp.T

    woe = np.zeros((65, 64), f)
    woe[0:64] = Wo.T
    woe[64] = bo_p

    w1e = np.zeros((65, 256), f)
    w1e[0:64] = W1p.T
    w1e[64] = b1p

    w2s = np.concatenate([W2.T[0:128], W2.T[128:256]], axis=1).astype(f)  # [128,128]

    return {
        "wsv": _bf16(wsv),
        "wps": _bf16(wps),
        "woe": _bf16(woe),
        "w1e": _bf16(w1e),
        "w2s": _bf16(w2s),
        "b2c": b2.astype(f),
        "ident": _bf16(np.eye(128, dtype=f)),
        "ones": _bf16(np.ones(128, f)),
    }


def _patch_tile_drain():
    """This container's walrus build rejects instructions with more than one
    sync-wait command. Tile's kernel-tail drain carries one wait per logical
    processor; split them across sync-engine nops."""
    import concourse.tile as tile
    from concourse.vector_clock import ScopedClock, VectorClock

    if getattr(tile.TileContext, "_ant_drain_patched", False):
        return

    def _drain_and_barrier(self, tick_clock, wait_clock):
        nc = self.nc
        gc = tick_clock.global_clock
        n = len(gc)
        procs = [i for i in range(n) if gc[i] > 0]
        for p in procs:
            sub = VectorClock([gc[j] if j == p else 0 for j in range(n)])
            nop = nc.sync.nop(nofuse=True, hint="drain_split")
            wait_clock.add_sem_waits(nop.ins, ScopedClock({None: sub}))
        nc.sync.drain()
        nc.all_engine_barrier()
        popped = nc._tile_sem_poison_stack.pop()
        assert popped is self._sem_poison
        nc.clear_and_free_semaphores(list(self.sems.allocated().values()))
        nc.all_engine_barrier()

    tile.TileContext._drain_and_barrier = _drain_and_barrier
    tile.TileContext._ant_drain_patched = True


def _split_multi_waits(nc):
    """Hoist extra sync waits onto same-engine NoOps inserted right before
    the instruction (the engine stalls at the nop, semantics unchanged)."""
    import bass_rust
    import concourse.mybir as mybir

    k = 0
    for blk in nc.main_func.blocks:
        insts = blk.instructions
        need = False
        for ins in insts:
            si = ins.sync_info
            if si is not None and len(si.on_wait) > 1:
                need = True
                break
        if not need:
            continue
        out = []
        for ins in insts:
            si = ins.sync_info
            if (
                si is not None
                and len(si.on_wait) > 1
                and ins.engine != mybir.EngineType.Unassigned
            ):
                waits = list(si.on_wait)
                for w in waits[:-1]:
                    k += 1
                    nop = mybir.InstNoOp(
                        name=f"wsplit-{k}", ins=[], outs=[], engine=ins.engine
                    )
                    nop.sync_info = bass_rust.SyncInfo(on_wait=[w], on_update=[])
                    nc.register_instruction(nop, overwrite=True)
                    out.append(nop)
                ins.sync_info = bass_rust.SyncInfo(
                    on_wait=[waits[-1]], on_update=list(si.on_update)
                )
            out.append(ins)
        blk.instructions = out


def build_program(n_tokens):
    """Build the per-core Bass program (same for every core)."""
    import concourse.bass as bass
    import concourse.tile as tile
    import concourse.mybir as mybir

    _patch_tile_drain()

    dt = mybir.dt
    f32 = dt.float32
    bf16 = dt.bfloat16
    Alu = mybir.AluOpType
    Act = mybir.ActivationFunctionType

    nc = bass.Bass(trn_type="TRN2")

    k_d = nc.dram_tensor("k", [n_tokens, M * D], f32, kind="ExternalInput")
    pos_d = nc.dram_tensor("pos", [n_tokens, M * 4], f32, kind="ExternalInput")
    q_d = nc.dram_tensor("q", [n_tokens, D], f32, kind="ExternalInput")
    wsv_d = nc.dram_tensor("wsv", [128, 256], bf16, kind="ExternalInput")
    wps_d = nc.dram_tensor("wps", [128, 1024], bf16, kind="ExternalInput")
    woe_d = nc.dram_tensor("woe", [65, 64], bf16, kind="ExternalInput")
    w1e_d = nc.dram_tensor("w1e", [65, 256], bf16, kind="ExternalInput")
    w2s_d = nc.dram_tensor("w2s", [128, 128], bf16, kind="ExternalInput")
    b2c_d = nc.dram_tensor("b2c", [D], f32, kind="ExternalInput")
    ident_d = nc.dram_tensor("ident", [128, 128], bf16, kind="ExternalInput")
    ones_d = nc.dram_tensor("ones", [128], bf16, kind="ExternalInput")
    out_d = nc.dram_tensor("out", [n_tokens, D], f32, kind="ExternalOutput")

    nchunks = n_tokens // CHUNK
    npairs = nchunks // 2
    assert n_tokens % (2 * CHUNK) == 0

    with tile.TileContext(nc) as tc:
        with (
            tc.tile_pool(name="singles", bufs=1) as singles,
            tc.tile_pool(name="kin", bufs=3) as kin,
            tc.tile_pool(name="small_in", bufs=6) as small_in,
            tc.tile_pool(name="ts", bufs=3) as tsp,
            tc.tile_pool(name="ev", bufs=3) as evp_pool,
            tc.tile_pool(name="lnw", bufs=8) as lnw,
            tc.tile_pool(name="outp", bufs=6) as outp,
            tc.tile_pool(name="sp", bufs=2, space="PSUM") as sp_pool,
            tc.tile_pool(name="tp_ps", bufs=2, space="PSUM") as tp_ps,
            tc.tile_pool(name="tail_ps", bufs=2, space="PSUM") as tail_ps,
        ):
            # constants
            WSV = singles.tile([128, 256], bf16)
            WPS = singles.tile([128, 1024], bf16)
            WOE = singles.tile([65, 64], bf16)
            W1E = singles.tile([65, 256], bf16)
            W2S = singles.tile([128, 128], bf16)
            B2B = singles.tile([128, 2, D], f32)
            IDENT = singles.tile([128, 128], bf16)
            EPST = singles.tile([128, 1], f32)
            nc.vector.memset(EPST[:], EPS)
            OTSX = [
                singles.tile([65, 128], bf16, tag=f"otsx{i}", name=f"OTSX{i}")
                for i in range(4)
            ]
            HTSX = [
                singles.tile([65, 128], bf16, tag=f"htsx{i}", name=f"HTSX{i}")
                for i in range(4)
            ]
            for t in OTSX:
                nc.sync.dma_start(out=t[64:65, :], in_=ones_d[:])
            for t in HTSX:
                nc.sync.dma_start(out=t[64:65, :], in_=ones_d[:])
            nc.sync.dma_start(out=WSV[:], in_=wsv_d[:])
            nc.sync.dma_start(out=WPS[:], in_=wps_d[:])
            nc.sync.dma_start(out=WOE[:], in_=woe_d[:])
            nc.sync.dma_start(out=W1E[:], in_=w1e_d[:])
            nc.sync.dma_start(out=W2S[:], in_=w2s_d[:])
            nc.sync.dma_start(out=IDENT[:], in_=ident_d[:])
            b2_bcast = bass.AP(
                tensor=b2c_d[:].tensor, offset=0, ap=[[0, 128], [0, 2], [1, D]]
            )
            nc.gpsimd.dma_start(out=B2B[:], in_=b2_bcast)

            fstate = {}

            def front(ci, E2, EV2):
                """Front stage for one 128-token chunk ci; writes its share
                of the pair tiles E2/EV2 (cc = ci%2)."""
                n0 = ci * CHUNK
                cc = ci % 2
                if cc == 0:
                    # 2-chunk batched loads (cast f32->bf16 during DMA)
                    KD2 = kin.tile([128, 2, M * D], bf16, tag="kd")
                    nc.gpsimd.dma_start(
                        out=KD2[:],
                        in_=k_d[n0 : n0 + 256, :].rearrange(
                            "(c p) e -> p c e", c=2, p=128
                        ),
                    )
                    PD2 = small_in.tile([128, 2, M * 4], bf16, tag="pd")
                    nc.gpsimd.dma_start(
                        out=PD2[:],
                        in_=pos_d[n0 : n0 + 256, :].rearrange(
                            "(c p) e -> p c e", c=2, p=128
                        ),
                    )
                    PT2 = tp_ps.tile([128, 128], bf16, tag="tp")
                    nc.tensor.transpose(
                        PT2[:], PD2[:].rearrange("p c e -> p (c e)"), IDENT[:]
                    )
                    PTS2 = tsp.tile([128, 128], bf16, tag="pts")
                    nc.scalar.copy(out=PTS2[:], in_=PT2[:])
                    fstate["KD2"] = KD2
                    fstate["PTS2"] = PTS2
                KD2 = fstate["KD2"]
                PTS2 = fstate["PTS2"]

                # transposed k tiles via PE into PSUM; evacuate per half so
                # the S/v matmuls of half 0 start while half 1 transposes
                TS = tsp.tile([128, M * D], bf16, tag="ts")
                for hh in range(2):
                    TPh = tp_ps.tile([128, 512], bf16, tag="tp")
                    for jj in range(4):
                        j = 4 * hh + jj
                        nc.tensor.transpose(
                            TPh[:, 128 * jj : 128 * (jj + 1)],
                            KD2[:, cc, 128 * j : 128 * (j + 1)],
                            IDENT[:],
                        )
                    nc.scalar.copy(
                        out=TS[:, 512 * hh : 512 * (hh + 1)], in_=TPh[:]
                    )

                for h in range(2):
                    # [t, bank, S|v, pair, parity, d]
                    SPh = sp_pool.tile([128, 2, 2, 2, 2, D], f32, tag="sp")
                    for j in range(4):
                        bk, jm = j // 2, j % 2
                        nc.tensor.matmul(
                            SPh[:, bk, :, jm, :, :],
                            TS[:, 128 * (4 * h + j) : 128 * (4 * h + j + 1)],
                            WSV[:],
                            start=(jm == 0),
                            stop=False,
                        )
                    for bk in range(2):
                        blk = 2 * h + bk
                        nc.tensor.matmul(
                            SPh[:, bk, 0, :, :, :],
                            PTS2[64 * cc : 64 * cc + 64, :],
                            WPS[64 * cc : 64 * cc + 64, 256 * blk : 256 * (blk + 1)],
                            start=False,
                            stop=True,
                        )
                    nc.scalar.activation(
                        out=E2[:, cc, h, :],
                        in_=SPh[:, :, 0, :, :, :].rearrange("p b j r d -> p d b j r"),
                        func=Act.Exp,
                    )
                    nc.vector.tensor_mul(
                        EV2[:, cc, h, :],
                        E2[:, cc, h, :],
                        SPh[:, :, 1, :, :, :].rearrange("p b j r d -> p d b j r"),
                    )

            def back1(pi, E2, EV2, QD2):
                # m-sums for both chunks of the pair; free layout (c, d)
                SM2 = lnw.tile([128, 2, D], f32, tag="sm")
                nc.vector.tensor_reduce(
                    out=SM2[:],
                    in_=E2[:].rearrange("p c h (d m) -> p c d h m", d=D, m=8),
                    axis=mybir.AxisListType.XY,
                    op=Alu.add,
                )
                OV1 = lnw.tile([128, 2, 512], bf16, tag="ov1")
                nc.gpsimd.tensor_add(OV1[:], EV2[:, :, 0, :], EV2[:, :, 1, :])
                OV1v = OV1[:].rearrange("p c (d m) -> p c d m", d=D, m=8)
                OV2a = lnw.tile([128, 2, D, 4], bf16, tag="ov2a")
                nc.gpsimd.tensor_add(OV2a[:], OV1v[:, :, :, 0:4], OV1v[:, :, :, 4:8])
                OV2 = lnw.tile([128, 2, D], f32, tag="ov")
                nc.vector.tensor_reduce(
                    out=OV2[:],
                    in_=OV2a[:],
                    axis=mybir.AxisListType.X,
                    op=Alu.add,
                )
                LS2 = lnw.tile([128, 2, D], f32, tag="ls")
                nc.scalar.activation(out=LS2[:], in_=SM2[:], func=Act.Ln)
                RC2 = lnw.tile([128, 2, D], f32, tag="rc")
                nc.scalar.activation(out=RC2[:], in_=LS2[:], func=Act.Exp, scale=-1.0)
                OA2 = lnw.tile([128, 2, D], bf16, tag="oa")
                nc.gpsimd.tensor_mul(OA2[:], OV2[:], RC2[:])
                return OA2, QD2

            def back2(pi, OA2, QD2):
                # Wo + residual for both chunks
                OT2 = tail_ps.tile([128, 128], bf16, tag="tlps")
                nc.tensor.transpose(
                    OT2[:], OA2[:].rearrange("p c d -> p (c d)"), IDENT[:]
                )
                OO2 = tail_ps.tile([128, 2, D], f32, tag="tlps")
                for cc in range(2):
                    OTS = OTSX[(2 * pi + cc) % 4]
                    nc.scalar.copy(out=OTS[0:64, :], in_=OT2[64 * cc : 64 * cc + 64, :])
                    nc.tensor.matmul(
                        OO2[:, cc, :], OTS[:], WOE[:], start=True, stop=True
                    )
                R12 = lnw.tile([128, 2, D], f32, tag="r1")
                nc.vector.tensor_add(R12[:], QD2[:], OO2[:])
                RB2 = lnw.tile([128, 2, D], f32, tag="rb")
                nc.gpsimd.tensor_add(RB2[:], R12[:], B2B[:])

                # LN2 stats per chunk; rstd = exp(-0.5*ln(var+eps)), batched
                MVA = lnw.tile([128, 2, 2], f32, tag="mva")
                for cc in range(2):
                    ST6 = lnw.tile([128, 6], f32, tag="st6")
                    nc.vector.bn_stats(out=ST6[:], in_=R12[:, cc, :])
                    nc.vector.bn_aggr(out=MVA[:, cc, :], in_=ST6[:])
                LV2 = lnw.tile([128, 2], f32, tag="lv")
                nc.scalar.activation(
                    out=LV2[:], in_=MVA[:, :, 1], func=Act.Ln, bias=EPST[:]
                )
                YA2 = lnw.tile([128, 2], f32, tag="ya")
                nc.scalar.activation(out=YA2[:], in_=LV2[:], func=Act.Exp, scale=-0.5)
                CT2 = lnw.tile([128, 2, D], bf16, tag="ct")
                for cc in range(2):
                    nc.vector.tensor_scalar(
                        out=CT2[:, cc, :],
                        in0=R12[:, cc, :],
                        scalar1=MVA[:, cc, 0:1],
                        scalar2=YA2[:, cc : cc + 1],
                        op0=Alu.subtract,
                        op1=Alu.mult,
                    )
                return CT2, RB2

            def back3(pi, CT2, RB2):
                n0 = 2 * pi * CHUNK
                # MLP (bf16); first layer computed directly transposed:
                # H1T[f,t] with W1 halves stationary, bias via ones row.
                HT2 = tail_ps.tile([128, 128], bf16, tag="tlps")
                nc.tensor.transpose(
                    HT2[:], CT2[:].rearrange("p c d -> p (c d)"), IDENT[:]
                )
                H1S = lnw.tile([128, 2, 2, 128], bf16, tag="h1s")
                H1T = [
                    tail_ps.tile([128, 2, 128], f32, tag="tlps", name=f"H1T{fh}")
                    for fh in range(2)
                ]
                for cc in range(2):
                    HTS = HTSX[(2 * pi + cc) % 4]
                    nc.scalar.copy(
                        out=HTS[0:64, :], in_=HT2[64 * cc : 64 * cc + 64, :]
                    )
                    for fh in range(2):
                        nc.tensor.matmul(
                            H1T[fh][:, cc, :],
                            W1E[:, 128 * fh : 128 * (fh + 1)],
                            HTS[:],
                            start=True,
                            stop=True,
                        )
                for fh in range(2):
                    nc.vector.tensor_scalar_max(
                        H1S[:, :, fh, :], H1T[fh][:], 0.0
                    )
                H22 = tail_ps.tile([128, 2, D], f32, tag="tlps")
                for cc in range(2):
                    for fh in range(2):
                        nc.tensor.matmul(
                            H22[:, cc, :],
                            H1S[:, cc, fh, :],
                            W2S[:, 64 * fh : 64 * fh + 64],
                            start=(fh == 0),
                            stop=(fh == 1),
                        )
                OUTT = outp.tile([128, 2, D], f32, tag="outt")
                nc.vector.tensor_add(OUTT[:], H22[:], RB2[:])
                nc.sync.dma_start(
                    out=out_d[n0 : n0 + 256, :].rearrange(
                        "(c p) d -> p c d", c=2, p=128
                    ),
                    in_=OUTT[:],
                )

            def load_front(pi):
                E2 = evp_pool.tile([128, 2, 2, 512], bf16, tag="e")
                EV2 = evp_pool.tile([128, 2, 2, 512], bf16, tag="evp")
                front(2 * pi, E2, EV2)
                front(2 * pi + 1, E2, EV2)
                QD2 = small_in.tile([128, 2, D], f32, tag="qd")
                nc.sync.dma_start(
                    out=QD2[:],
                    in_=q_d[2 * pi * CHUNK : 2 * pi * CHUNK + 256, :].rearrange(
                        "(c p) d -> p c d", c=2, p=128
                    ),
                )
                return E2, EV2, QD2

            L1, L2, L3 = 2, 1, 1
            pend1 = {}
            pend2 = {}
            pend3 = {}
            for pi in range(npairs + L1 + L2 + L3):
                if pi < npairs:
                    pend1[pi] = load_front(pi)
                c = pi - L1
                if 0 <= c < npairs:
                    pend2[c] = back1(c, *pend1.pop(c))
                c = pi - L1 - L2
                if 0 <= c < npairs:
                    pend3[c] = back2(c, *pend2.pop(c))
                c = pi - L1 - L2 - L3
                if 0 <= c < npairs:
                    back3(c, *pend3.pop(c))

    _split_multi_waits(nc)
    return nc


def _get_program(n_tokens):
    if n_tokens not in _prog_cache:
        _prog_cache[n_tokens] = build_program(n_tokens)
    return _prog_cache[n_tokens]


def make_in_maps(inputs):
    """Shard full inputs into 8 per-core input maps."""
    w = _prep_weights(inputs)
    k = np.ascontiguousarray(inputs["k"])
    pos = np.ascontiguousarray(inputs["pos"])
    q = np.ascontiguousarray(inputs["q"])
    nt = k.shape[1]
    in_maps = []
    for b in range(B):
        in_maps.append(
            {
                "k": k[b].reshape(nt, M * D),
                "pos": pos[b].reshape(nt, M * 4),
                "q": q[b].reshape(nt, D),
                **w,
            }
        )
    return in_maps


LAST_EXEC_NS = None
LAST_RESULT = None


def _install_cc_probe():
    import subprocess
    import traceback

    import libneuronxla

    if getattr(libneuronxla, "_ant_probe", False):
        return
    shim = libneuronxla.neuronx_cc

    def loud(code, *a, **k):
        try:
            return shim(code, *a, **k)
        except subprocess.CalledProcessError as e:
            with open("/tmp/walrus_err.log", "w") as fh:
                fh.write(str(e.output))
            raise
        except BaseException:
            with open("/tmp/walrus_err.log", "w") as fh:
                fh.write(traceback.format_exc())
            raise

    libneuronxla.neuronx_cc = loud
    libneuronxla._ant_probe = True
    import concourse.bass2jax as b2j

    b2j.install_neuronx_cc_hook = lambda: None


def _ensure_ntff_hook():
    """Register the NTFF profiling hook if the image's antenv lacks it."""
    import sys
    import types

    try:
        from antenv.axon_hooks import get_axon_ntff_profile_hook  # noqa: F401

        return
    except ImportError:
        pass
    try:
        from trn_agent_boot.trn_boot import _ntff_profile_via_ctypes

        hook = _ntff_profile_via_ctypes("/opt/axon/libaxon_pjrt.so")
    except Exception:
        hook = None
    mod = types.ModuleType("antenv.axon_hooks")
    mod.get_axon_ntff_profile_hook = lambda: hook
    mod.set_axon_ntff_profile_hook = lambda h: None
    import antenv

    sys.modules["antenv.axon_hooks"] = mod
    antenv.axon_hooks = mod


def kernel(**inputs):
    global LAST_EXEC_NS, LAST_RESULT
    import os

    from concourse import bass_utils

    _install_cc_probe()
    trace = bool(int(os.environ.get("KERNEL_TRACE", "0")))
    if trace:
        _ensure_ntff_hook()
    nt = np.ascontiguousarray(inputs["k"]).shape[1]
    nc = _get_program(nt)
    in_maps = make_in_maps(inputs)
    res = bass_utils.run_bass_kernel_spmd(
        nc, in_maps, core_ids=list(range(B)), trace=trace
    )
    LAST_EXEC_NS = res.exec_time_ns
    LAST_RESULT = res
    out = np.stack([res.results[b]["out"].reshape(nt, D) for b in range(B)])
    return out.astype(np.float32)


# revision 27
# speedup vs baseline: 1.9633x; 1.0069x over previous
"""Trainium2 Bass kernel for nn_CustomTransformer2D (gnn message passing).

Math (validated against the reference in fp64):
  - The q/LN1/Wq branch only shifts attention logits by a constant along the
    softmax axis M, so it cancels in softmax; q enters only via the residual.
  - bk and bp are constant along M too -> dropped from the logits.
  - v = Wv@(Wk@k+bk)+bv = k@(Wv@Wk)^T + bkv;  since sum_m a = 1, the bkv
    offset commutes through the attention sum and is folded into Wo's bias.
  - ln2 gain/bias are folded into W1/b1.

Per-core layout (batch b on core b): tokens on SBUF partitions; the matmul
path runs in bf16 (inputs cast during the SWDGE DMA), accumulation f32.

Per chunk of 128 tokens: k tile (2-chunk batched loads) --PE transpose per
m-pair--> TS; S/v matmuls stream WSV per transposed tile into a
(bank, S|v, pair, parity, d) PSUM layout so the pos matmuls land as 4
contiguous 256-col accumulations.  Softmax: exp on ACT (bf16 out), e*v on
DVE; all small downstream ops run at 2-chunk granularity to amortize
per-instruction overheads: single strided reduces for the m-sums, softmax
normalize via one GpSimd divide, LN2 rstd via exp(-0.5*ln(var+eps)) on ACT
(exp/ln/copy share one table set), MLP first layer computed directly in
transposed form (weights stationary, bias via the ones-row trick), relu on
DVE, second layer straight from the transposed hidden tile.
"""

import numpy as np

B, N, M, D, F = 8, 16384, 16, 64, 256
EPS = 1e-5
CHUNK = 128  # tokens per chunk

_prog_cache = {}


def _bf16(x):
    import ml_dtypes

    return np.asarray(x).astype(ml_dtypes.bfloat16)


def _prep_weights(inp):
    f = np.float32
    Wk, bk = inp["Wk"], inp["bk"]
    Wv, bv = inp["Wv"], inp["bv"]
    Wp = inp["Wp"]
    Wo, bo = inp["Wo"], inp["bo"]
    W1, b1 = inp["W1"], inp["b1"]
    W2, b2 = inp["W2"], inp["b2"]
    g2, bl2 = inp["ln2_g"], inp["ln2_b"]

    Wkv = (Wv @ Wk).astype(f)
    bkv = (Wv @ bk + bv).astype(f)
    bo_p = (bo + Wo @ bkv).astype(f)
    W1p = (W1 * g2[None, :]).astype(f)
    b1p = (b1 + W1 @ bl2).astype(f)

    wsv = np.zeros((128, 256), f)
    wsv[0:64, 0:64] = Wk.T
    wsv[64:128, 64:128] = Wk.T
    wsv[0:64, 128:192] = Wkv.T
    wsv[64:128, 192:256] = Wkv.T

    # pos weights, S columns only.  Per half h and bank bk the pos matmul
    # accumulates a contiguous 256-col block (jm, r, d) onto the S region;
    # block index = 2*h + bk, pair j = 4*h + 2*bk + jm, m = 2*j + r.
    # Duplicated on partition halves so the stationary PTS2 half (base
    # partition 0 or 64) matches the rhs base partition.
    wps = np.zeros((128, 1024), f)
    for blk in range(4):
        for jm in range(2):
            for r in range(2):
                m = 2 * (2 * blk + jm) + r
                for par in range(2):
                    rows = slice(64 * par + 4 * m, 64 * par + 4 * m + 4)
                    cols = slice(
                        256 * blk + 128 * jm + 64 * r,
                        256 * blk + 128 * jm + 64 * r + 64,
                    )
                    wps[rows, cols] = Wp.T

    woe = np.zeros((65, 64), f)
    woe[0:64] = Wo.T
    woe[64] = bo_p

    w1e = np.zeros((65, 256), f)
    w1e[0:64] = W1p.T
    w1e[64] = b1p

    w2s = np.concatenate([W2.T[0:128], W2.T[128:256]], axis=1).astype(f)  # [128,128]

    return {
        "wsv": _bf16(wsv),
        "wps": _bf16(wps),
        "woe": _bf16(woe),
        "w1e": _bf16(w1e),
        "w2s": _bf16(w2s),
        "b2c": b2.astype(f),
        "ident": _bf16(np.eye(128, dtype=f)),
        "ones": _bf16(np.ones(128, f)),
    }


def _patch_tile_drain():
    """This container's walrus build rejects instructions with more than one
    sync-wait command. Tile's kernel-tail drain carries one wait per logical
    processor; split them across sync-engine nops."""
    import concourse.tile as tile
    from concourse.vector_clock import ScopedClock, VectorClock

    if getattr(tile.TileContext, "_ant_drain_patched", False):
        return

    def _drain_and_barrier(self, tick_clock, wait_clock):
        nc = self.nc
        gc = tick_clock.global_clock
        n = len(gc)
        procs = [i for i in range(n) if gc[i] > 0]
        for p in procs:
            sub = VectorClock([gc[j] if j == p else 0 for j in range(n)])
            nop = nc.sync.nop(nofuse=True, hint="drain_split")
            wait_clock.add_sem_waits(nop.ins, ScopedClock({None: sub}))
        nc.sync.drain()
        nc.all_engine_barrier()
        popped = nc._tile_sem_poison_stack.pop()
        assert popped is self._sem_poison
        nc.clear_and_free_semaphores(list(self.sems.allocated().values()))
        nc.all_engine_barrier()

    tile.TileContext._drain_and_barrier = _drain_and_barrier
    tile.TileContext._ant_drain_patched = True


def _split_multi_waits(nc):
    """Hoist extra sync waits onto same-engine NoOps inserted right before
    the instruction (the engine stalls at the nop, semantics unchanged)."""
    import bass_rust
    import concourse.mybir as mybir

    k = 0
    for blk in nc.main_func.blocks:
        insts = blk.instructions
        need = False
        for ins in insts:
            si = ins.sync_info
            if si is not None and len(si.on_wait) > 1:
                need = True
                break
        if not need:
            continue
        out = []
        for ins in insts:
            si = ins.sync_info
            if (
                si is not None
                and len(si.on_wait) > 1
                and ins.engine != mybir.EngineType.Unassigned
            ):
                waits = list(si.on_wait)
                for w in waits[:-1]:
                    k += 1
                    nop = mybir.InstNoOp(
                        name=f"wsplit-{k}", ins=[], outs=[], engine=ins.engine
                    )
                    nop.sync_info = bass_rust.SyncInfo(on_wait=[w], on_update=[])
                    nc.register_instruction(nop, overwrite=True)
                    out.append(nop)
                ins.sync_info = bass_rust.SyncInfo(
                    on_wait=[waits[-1]], on_update=list(si.on_update)
                )
            out.append(ins)
        blk.instructions = out


def build_program(n_tokens):
    """Build the per-core Bass program (same for every core)."""
    import concourse.bass as bass
    import concourse.tile as tile
    import concourse.mybir as mybir

    _patch_tile_drain()

    dt = mybir.dt
    f32 = dt.float32
    bf16 = dt.bfloat16
    Alu = mybir.AluOpType
    Act = mybir.ActivationFunctionType

    nc = bass.Bass(trn_type="TRN2")

    k_d = nc.dram_tensor("k", [n_tokens, M * D], f32, kind="ExternalInput")
    pos_d = nc.dram_tensor("pos", [n_tokens, M * 4], f32, kind="ExternalInput")
    q_d = nc.dram_tensor("q", [n_tokens, D], f32, kind="ExternalInput")
    wsv_d = nc.dram_tensor("wsv", [128, 256], bf16, kind="ExternalInput")
    wps_d = nc.dram_tensor("wps", [128, 1024], bf16, kind="ExternalInput")
    woe_d = nc.dram_tensor("woe", [65, 64], bf16, kind="ExternalInput")
    w1e_d = nc.dram_tensor("w1e", [65, 256], bf16, kind="ExternalInput")
    w2s_d = nc.dram_tensor("w2s", [128, 128], bf16, kind="ExternalInput")
    b2c_d = nc.dram_tensor("b2c", [D], f32, kind="ExternalInput")
    ident_d = nc.dram_tensor("ident", [128, 128], bf16, kind="ExternalInput")
    ones_d = nc.dram_tensor("ones", [128], bf16, kind="ExternalInput")
    out_d = nc.dram_tensor("out", [n_tokens, D], f32, kind="ExternalOutput")

    nchunks = n_tokens // CHUNK
    npairs = nchunks // 2
    assert n_tokens % (2 * CHUNK) == 0

    with tile.TileContext(nc) as tc:
        with (
            tc.tile_pool(name="singles", bufs=1) as singles,
            tc.tile_pool(name="kin", bufs=3) as kin,
            tc.tile_pool(name="small_in", bufs=8) as small_in,
            tc.tile_pool(name="ts", bufs=3) as tsp,
            tc.tile_pool(name="ev", bufs=3) as evp_pool,
            tc.tile_pool(name="lnw", bufs=8) as lnw,
            tc.tile_pool(name="outp", bufs=6) as outp,
            tc.tile_pool(name="sp", bufs=2, space="PSUM") as sp_pool,
            tc.tile_pool(name="tp_ps", bufs=2, space="PSUM") as tp_ps,
            tc.tile_pool(name="tail_ps", bufs=2, space="PSUM") as tail_ps,
        ):
            # constants
            WSV = singles.tile([128, 256], bf16)
            WPS = singles.tile([128, 1024], bf16)
            WOE = singles.tile([65, 64], bf16)
            W1E = singles.tile([65, 256], bf16)
            W2S = singles.tile([128, 128], bf16)
            B2B = singles.tile([128, 2, D], f32)
            IDENT = singles.tile([128, 128], bf16)
            EPST = singles.tile([128, 1], f32)
            nc.vector.memset(EPST[:], EPS)
            OTSX = [
                singles.tile([65, 128], bf16, tag=f"otsx{i}", name=f"OTSX{i}")
                for i in range(4)
            ]
            HTSX = [
                singles.tile([65, 128], bf16, tag=f"htsx{i}", name=f"HTSX{i}")
                for i in range(4)
            ]
            for t in OTSX:
                nc.sync.dma_start(out=t[64:65, :], in_=ones_d[:])
            for t in HTSX:
                nc.sync.dma_start(out=t[64:65, :], in_=ones_d[:])
            nc.sync.dma_start(out=WSV[:], in_=wsv_d[:])
            nc.sync.dma_start(out=WPS[:], in_=wps_d[:])
            nc.sync.dma_start(out=WOE[:], in_=woe_d[:])
            nc.sync.dma_start(out=W1E[:], in_=w1e_d[:])
            nc.sync.dma_start(out=W2S[:], in_=w2s_d[:])
            nc.sync.dma_start(out=IDENT[:], in_=ident_d[:])
            b2_bcast = bass.AP(
                tensor=b2c_d[:].tensor, offset=0, ap=[[0, 128], [0, 2], [1, D]]
            )
            nc.gpsimd.dma_start(out=B2B[:], in_=b2_bcast)

            fstate = {}

            def prefetch(pi):
                """Issue the pair's input DMAs one pipeline step early so
                descriptor generation and the transfers overlap the
                previous pair's compute."""
                n0 = 2 * pi * CHUNK
                KD2 = kin.tile([128, 2, M * D], bf16, tag="kd")
                nc.gpsimd.dma_start(
                    out=KD2[:],
                    in_=k_d[n0 : n0 + 256, :].rearrange(
                        "(c p) e -> p c e", c=2, p=128
                    ),
                )
                PD2 = small_in.tile([128, 2, M * 4], bf16, tag="pd")
                nc.gpsimd.dma_start(
                    out=PD2[:],
                    in_=pos_d[n0 : n0 + 256, :].rearrange(
                        "(c p) e -> p c e", c=2, p=128
                    ),
                )
                QD2 = small_in.tile([128, 2, D], f32, tag="qd")
                nc.sync.dma_start(
                    out=QD2[:],
                    in_=q_d[n0 : n0 + 256, :].rearrange(
                        "(c p) d -> p c d", c=2, p=128
                    ),
                )
                return KD2, PD2, QD2

            def front(ci, E2, EV2, KD2, PD2):
                """Front stage for one 128-token chunk ci; writes its share
                of the pair tiles E2/EV2 (cc = ci%2)."""
                cc = ci % 2
                if cc == 0:
                    PT2 = tp_ps.tile([128, 128], bf16, tag="tp")
                    nc.tensor.transpose(
                        PT2[:], PD2[:].rearrange("p c e -> p (c e)"), IDENT[:]
                    )
                    PTS2 = tsp.tile([128, 128], bf16, tag="pts")
                    nc.scalar.copy(out=PTS2[:], in_=PT2[:])
                    fstate["PTS2"] = PTS2
                PTS2 = fstate["PTS2"]

                # transposed k tiles via PE into PSUM; evacuate per half so
                # the S/v matmuls of half 0 start while half 1 transposes
                TS = tsp.tile([128, M * D], bf16, tag="ts")
                for hh in range(2):
                    TPh = tp_ps.tile([128, 512], bf16, tag="tp")
                    for jj in range(4):
                        j = 4 * hh + jj
                        nc.tensor.transpose(
                            TPh[:, 128 * jj : 128 * (jj + 1)],
                            KD2[:, cc, 128 * j : 128 * (j + 1)],
                            IDENT[:],
                        )
                    nc.scalar.copy(
                        out=TS[:, 512 * hh : 512 * (hh + 1)], in_=TPh[:]
                    )

                for h in range(2):
                    # [t, bank, S|v, pair, parity, d]
                    SPh = sp_pool.tile([128, 2, 2, 2, 2, D], f32, tag="sp")
                    for j in range(4):
                        bk, jm = j // 2, j % 2
                        nc.tensor.matmul(
                            SPh[:, bk, :, jm, :, :],
                            TS[:, 128 * (4 * h + j) : 128 * (4 * h + j + 1)],
                            WSV[:],
                            start=(jm == 0),
                            stop=False,
                        )
                    for bk in range(2):
                        blk = 2 * h + bk
                        nc.tensor.matmul(
                            SPh[:, bk, 0, :, :, :],
                            PTS2[64 * cc : 64 * cc + 64, :],
                            WPS[64 * cc : 64 * cc + 64, 256 * blk : 256 * (blk + 1)],
                            start=False,
                            stop=True,
                        )
                    nc.scalar.activation(
                        out=E2[:, cc, h, :],
                        in_=SPh[:, :, 0, :, :, :].rearrange("p b j r d -> p d b j r"),
                        func=Act.Exp,
                    )
                    nc.vector.tensor_mul(
                        EV2[:, cc, h, :],
                        E2[:, cc, h, :],
                        SPh[:, :, 1, :, :, :].rearrange("p b j r d -> p d b j r"),
                    )

            def back1(pi, E2, EV2, QD2):
                # m-sums for both chunks of the pair; free layout (c, d)
                SM2 = lnw.tile([128, 2, D], f32, tag="sm")
                nc.vector.tensor_reduce(
                    out=SM2[:],
                    in_=E2[:].rearrange("p c h (d m) -> p c d h m", d=D, m=8),
                    axis=mybir.AxisListType.XY,
                    op=Alu.add,
                )
                OV1 = lnw.tile([128, 2, 512], bf16, tag="ov1")
                nc.gpsimd.tensor_add(OV1[:], EV2[:, :, 0, :], EV2[:, :, 1, :])
                OV1v = OV1[:].rearrange("p c (d m) -> p c d m", d=D, m=8)
                OV2a = lnw.tile([128, 2, D, 4], bf16, tag="ov2a")
                nc.gpsimd.tensor_add(OV2a[:], OV1v[:, :, :, 0:4], OV1v[:, :, :, 4:8])
                OV2 = lnw.tile([128, 2, D], f32, tag="ov")
                nc.vector.tensor_reduce(
                    out=OV2[:],
                    in_=OV2a[:],
                    axis=mybir.AxisListType.X,
                    op=Alu.add,
                )
                LS2 = lnw.tile([128, 2, D], f32, tag="ls")
                nc.scalar.activation(out=LS2[:], in_=SM2[:], func=Act.Ln)
                RC2 = lnw.tile([128, 2, D], f32, tag="rc")
                nc.scalar.activation(out=RC2[:], in_=LS2[:], func=Act.Exp, scale=-1.0)
                OA2 = lnw.tile([128, 2, D], bf16, tag="oa")
                nc.gpsimd.tensor_mul(OA2[:], OV2[:], RC2[:])
                return OA2, QD2

            def back2(pi, OA2, QD2):
                # Wo + residual for both chunks
                OT2 = tail_ps.tile([128, 128], bf16, tag="tlps")
                nc.tensor.transpose(
                    OT2[:], OA2[:].rearrange("p c d -> p (c d)"), IDENT[:]
                )
                OO2 = tail_ps.tile([128, 2, D], f32, tag="tlps")
                for cc in range(2):
                    OTS = OTSX[(2 * pi + cc) % 4]
                    nc.scalar.copy(out=OTS[0:64, :], in_=OT2[64 * cc : 64 * cc + 64, :])
                    nc.tensor.matmul(
                        OO2[:, cc, :], OTS[:], WOE[:], start=True, stop=True
                    )
                R12 = lnw.tile([128, 2, D], f32, tag="r1")
                nc.vector.tensor_add(R12[:], QD2[:], OO2[:])
                RB2 = lnw.tile([128, 2, D], f32, tag="rb")
                nc.gpsimd.tensor_add(RB2[:], R12[:], B2B[:])

                # LN2 stats per chunk; rstd = exp(-0.5*ln(var+eps)), batched
                MVA = lnw.tile([128, 2, 2], f32, tag="mva")
                for cc in range(2):
                    ST6 = lnw.tile([128, 6], f32, tag="st6")
                    nc.vector.bn_stats(out=ST6[:], in_=R12[:, cc, :])
                    nc.vector.bn_aggr(out=MVA[:, cc, :], in_=ST6[:])
                LV2 = lnw.tile([128, 2], f32, tag="lv")
                nc.scalar.activation(
                    out=LV2[:], in_=MVA[:, :, 1], func=Act.Ln, bias=EPST[:]
                )
                YA2 = lnw.tile([128, 2], f32, tag="ya")
                nc.scalar.activation(out=YA2[:], in_=LV2[:], func=Act.Exp, scale=-0.5)
                CT2 = lnw.tile([128, 2, D], bf16, tag="ct")
                for cc in range(2):
                    nc.vector.tensor_scalar(
                        out=CT2[:, cc, :],
                        in0=R12[:, cc, :],
                        scalar1=MVA[:, cc, 0:1],
                        scalar2=YA2[:, cc : cc + 1],
                        op0=Alu.subtract,
                        op1=Alu.mult,
                    )
                return CT2, RB2

            def back3(pi, CT2, RB2):
                n0 = 2 * pi * CHUNK
                # MLP (bf16); first layer computed directly transposed:
                # H1T[f,t] with W1 halves stationary, bias via ones row.
                HT2 = tail_ps.tile([128, 128], bf16, tag="tlps")
                nc.tensor.transpose(
                    HT2[:], CT2[:].rearrange("p c d -> p (c d)"), IDENT[:]
                )
                H1S = lnw.tile([128, 2, 2, 128], bf16, tag="h1s")
                H1T = [
                    tail_ps.tile([128, 2, 128], f32, tag="tlps", name=f"H1T{fh}")
                    for fh in range(2)
                ]
                for cc in range(2):
                    HTS = HTSX[(2 * pi + cc) % 4]
                    nc.scalar.copy(
                        out=HTS[0:64, :], in_=HT2[64 * cc : 64 * cc + 64, :]
                    )
                    for fh in range(2):
                        nc.tensor.matmul(
                            H1T[fh][:, cc, :],
                            W1E[:, 128 * fh : 128 * (fh + 1)],
                            HTS[:],
                            start=True,
                            stop=True,
                        )
                for fh in range(2):
                    nc.vector.tensor_scalar_max(
                        H1S[:, :, fh, :], H1T[fh][:], 0.0
                    )
                H22 = tail_ps.tile([128, 2, D], f32, tag="tlps")
                for cc in range(2):
                    for fh in range(2):
                        nc.tensor.matmul(
                            H22[:, cc, :],
                            H1S[:, cc, fh, :],
                            W2S[:, 64 * fh : 64 * fh + 64],
                            start=(fh == 0),
                            stop=(fh == 1),
                        )
                OUTT = outp.tile([128, 2, D], f32, tag="outt")
                nc.vector.tensor_add(OUTT[:], H22[:], RB2[:])
                nc.sync.dma_start(
                    out=out_d[n0 : n0 + 256, :].rearrange(
                        "(c p) d -> p c d", c=2, p=128
                    ),
                    in_=OUTT[:],
                )

            def load_front(pi, KD2, PD2, QD2):
                E2 = evp_pool.tile([128, 2, 2, 512], bf16, tag="e")
                EV2 = evp_pool.tile([128, 2, 2, 512], bf16, tag="evp")
                front(2 * pi, E2, EV2, KD2, PD2)
                front(2 * pi + 1, E2, EV2, KD2, PD2)
                return E2, EV2, QD2

            L1, L2, L3 = 2, 1, 1
            pref = {}
            pend1 = {}
            pend2 = {}
            pend3 = {}
            for pi in range(npairs + L1 + L2 + L3):
                for tgt in (pi, pi + 1):
                    if tgt < npairs and tgt not in pref:
                        pref[tgt] = prefetch(tgt)
                if pi < npairs:
                    pend1[pi] = load_front(pi, *pref.pop(pi))
                c = pi - L1
                if 0 <= c < npairs:
                    pend2[c] = back1(c, *pend1.pop(c))
                c = pi - L1 - L2
                if 0 <= c < npairs:
                    pend3[c] = back2(c, *pend2.pop(c))
                c = pi - L1 - L2 - L3
                if 0 <= c < npairs:
                    back3(c, *pend3.pop(c))

    _split_multi_waits(nc)
    return nc


def _get_program(n_tokens):
    if n_tokens not in _prog_cache:
        _prog_cache[n_tokens] = build_program(n_tokens)
    return _prog_cache[n_tokens]


def make_in_maps(inputs):
    """Shard full inputs into 8 per-core input maps."""
    w = _prep_weights(inputs)
    k = np.ascontiguousarray(inputs["k"])
    pos = np.ascontiguousarray(inputs["pos"])
    q = np.ascontiguousarray(inputs["q"])
    nt = k.shape[1]
    in_maps = []
    for b in range(B):
        in_maps.append(
            {
                "k": k[b].reshape(nt, M * D),
                "pos": pos[b].reshape(nt, M * 4),
                "q": q[b].reshape(nt, D),
                **w,
            }
        )
    return in_maps


LAST_EXEC_NS = None
LAST_RESULT = None


def _install_cc_probe():
    import subprocess
    import traceback

    import libneuronxla

    if getattr(libneuronxla, "_ant_probe", False):
        return
    shim = libneuronxla.neuronx_cc

    def loud(code, *a, **k):
        try:
            return shim(code, *a, **k)
        except subprocess.CalledProcessError as e:
            with open("/tmp/walrus_err.log", "w") as fh:
                fh.write(str(e.output))
            raise
        except BaseException:
            with open("/tmp/walrus_err.log", "w") as fh:
                fh.write(traceback.format_exc())
            raise

    libneuronxla.neuronx_cc = loud
    libneuronxla._ant_probe = True
    import concourse.bass2jax as b2j

    b2j.install_neuronx_cc_hook = lambda: None


def _ensure_ntff_hook():
    """Register the NTFF profiling hook if the image's antenv lacks it."""
    import sys
    import types

    try:
        from antenv.axon_hooks import get_axon_ntff_profile_hook  # noqa: F401

        return
    except ImportError:
        pass
    try:
        from trn_agent_boot.trn_boot import _ntff_profile_via_ctypes

        hook = _ntff_profile_via_ctypes("/opt/axon/libaxon_pjrt.so")
    except Exception:
        hook = None
    mod = types.ModuleType("antenv.axon_hooks")
    mod.get_axon_ntff_profile_hook = lambda: hook
    mod.set_axon_ntff_profile_hook = lambda h: None
    import antenv

    sys.modules["antenv.axon_hooks"] = mod
    antenv.axon_hooks = mod


def kernel(**inputs):
    global LAST_EXEC_NS, LAST_RESULT
    import os

    from concourse import bass_utils

    _install_cc_probe()
    trace = bool(int(os.environ.get("KERNEL_TRACE", "0")))
    if trace:
        _ensure_ntff_hook()
    nt = np.ascontiguousarray(inputs["k"]).shape[1]
    nc = _get_program(nt)
    in_maps = make_in_maps(inputs)
    res = bass_utils.run_bass_kernel_spmd(
        nc, in_maps, core_ids=list(range(B)), trace=trace
    )
    LAST_EXEC_NS = res.exec_time_ns
    LAST_RESULT = res
    out = np.stack([res.results[b]["out"].reshape(nt, D) for b in range(B)])
    return out.astype(np.float32)
